# revision 10
# baseline (speedup 1.0000x reference)
"""Multi-head attention (16 heads, S=2048, d_model=1024, d_head=64) on 8 TRN2
NeuronCores, tensor-parallel over heads (2 heads per core).

Restructured from the 120us baseline around the TimelineSim cost model, where
matmul cost = output-free-size rows (K and M are free):

  * AV matmuls run transposed: out[sq=128, dv+1=65] with the exp tile as the
    stationary operand, 65 rows per accumulation step instead of 512 -> PE
    drops from 196k to 166k rows (~82us -> ~69us busy).
  * softmax normalize becomes a per-partition tensor_scalar (denominator is
    column 64 of the accumulator) -- no gpsimd broadcast.
  * z PSUM tiles are [128, 1024] (2 banks); exp runs 64x instead of 128x,
    halving the fixed per-instruction ACT overhead (~81us -> ~66us busy).
  * everything 2-byte: fp16 in/out of every matmul, exp computes
    exp(z/8 - 4) so the scores fit fp16 (max z/8 ~ 11.9), the bias cancels
    in the normalize. Output ships fp16 (half the out DMA), host sums in f32.
  * heads come out of AV as [sq, dv]; a 16x[128,128] PE transpose (+DVE copy)
    restores [hd, sq] for the output projection.

Schedule: inputs stream K0,Q0,K1,Q1,V0,K2,Q2,V1,K3,Q3,V2,V3 so the exp
stream (the ~66us ACT critical path, the pacer) starts by ~7us and never
starves. z/exp tiles are emitted greedily (group-ascending); group 0's AV
rides inline behind the V projections; groups 1-3 accumulate at the end of
the stream in group order, each followed by its normalize/transpose/outproj/
DMA chain so only group 3's chain is a tail. PSUM: 2x[128,1024] z +
2x[128,260] AV accumulators + 2x[128,512] proj/outproj = exactly 8 banks.
"""

import os

import numpy as np

import concourse.bass as bass
import concourse.tile as tile
from concourse import bacc, mybir
from concourse.bass_utils import run_bass_kernel_spmd

HEADS, D_K, D_V, D_X, D_M, S = 16, 64, 64, 1024, 1024, 2048
NCORES = 8
HPC = HEADS // NCORES          # heads per core
HD = HPC * D_K                 # 128: stacked head dim per core
SQW = 512                      # sq group width
NSQ = S // SQW                 # 4 groups
SKW = 128                      # sk chunk width (partition dim)
NSK = S // SKW                 # 16
NXC = D_X // 128               # 8 contraction chunks for projections
NJJ = NSK // 2                 # 8 skc-pairs (one [128,1024] z tile each)

F32 = mybir.dt.float32
F16 = mybir.dt.float16
EXP = mybir.ActivationFunctionType.Exp

LAST_EXEC_NS = None
_NC_CACHE = None


def _emit(tc, nc, aps):
    from contextlib import ExitStack

    qt, kt, vt, wq, wk, wv, wot, ident, out = (
        aps["qt"], aps["kt"], aps["vt"], aps["wq"], aps["wk"], aps["wv"],
        aps["wot"], aps["ident"], aps["out"],
    )

    with ExitStack() as ctx:
        wpool = ctx.enter_context(tc.tile_pool(name="weights", bufs=1))
        proj = ctx.enter_context(tc.tile_pool(name="proj", bufs=1))
        inp = ctx.enter_context(tc.tile_pool(name="inp", bufs=6))
        etp = ctx.enter_context(tc.tile_pool(name="et", bufs=52))
        hsqp = ctx.enter_context(tc.tile_pool(name="hsq", bufs=8))
        outp = ctx.enter_context(tc.tile_pool(name="outs", bufs=4))
        smalls = ctx.enter_context(tc.tile_pool(name="smalls", bufs=4))
        ps_z = ctx.enter_context(tc.tile_pool(name="ps_z", bufs=2, space="PSUM"))
        ps_av = ctx.enter_context(tc.tile_pool(name="ps_av", bufs=2, space="PSUM"))
        ps_pr = ctx.enter_context(tc.tile_pool(name="ps_pr", bufs=2, space="PSUM"))

        # ---- persistent SBUF tensors ----
        wq_sb = wpool.tile([128, D_X], F16, tag="wq")     # (xc p) stacked chunks
        wk_sb = wpool.tile([128, D_X], F16, tag="wk")
        wv_sb = wpool.tile([128, D_X], F16, tag="wv")
        wot_sb = wpool.tile([HD, D_M], F16, tag="wot")
        ident_sb = wpool.tile([128, 128], F16, tag="ident")
        qpt_sb = proj.tile([HD, S], F16, tag="qpt")
        kpt_sb = proj.tile([HD, S], F16, tag="kpt")
        # VpAug: per (h, skc) a (128 sk, 65) block: cols 0-63 = Vp, col 64 = 1
        vpa_sb = proj.tile([128, HPC * NSK * 65], F16, tag="vpa")
        headst_sb = proj.tile([HD, S], F16, tag="headst")

        def load_w(w_dram, w_sb):
            nc.sync.dma_start(w_sb[:], w_dram)

        def load_chunk(tt_dram, c, name):
            """One DMA: all 8 xc strips of a 512-wide chunk -> (128, 8, 512)."""
            t = inp.tile([128, NXC, SQW], F16, tag="inp", name=name)
            nc.sync.dma_start(
                t[:],
                tt_dram.rearrange("(xc p) s -> p xc s", p=128)[
                    :, :, c * SQW:(c + 1) * SQW
                ],
            )
            return t

        def project(t, w_sb, dst_sb, c, name):
            """dst_sb[:, c*512:(c+1)*512] = W.T @ X.T chunk (fp16)."""
            ps = ps_pr.tile([128, SQW], F32, tag="pr", name=name)
            for xc in range(NXC):
                nc.tensor.matmul(
                    ps[:],
                    w_sb[:, xc * 128:(xc + 1) * 128],
                    t[:, xc, :],
                    start=(xc == 0),
                    stop=(xc == NXC - 1),
                )
            nc.vector.tensor_copy(dst_sb[:, c * SQW:(c + 1) * SQW], ps[:])

        def project_v(t, c):
            """VpAug sk-chunks for 512-chunk c: Vp = VT_chunk.T @ Wv directly
            in (sk, hd) layout."""
            for j in range(SQW // SKW):
                skc = c * (SQW // SKW) + j
                ps = ps_pr.tile([128, HD], F32, tag="pr", name=f"vp_{skc}",
                                padded_shape=[128, SQW])
                for xc in range(NXC):
                    nc.tensor.matmul(
                        ps[:],
                        t[:, xc, j * SKW:(j + 1) * SKW],
                        wv_sb[:, xc * 128:(xc + 1) * 128],
                        start=(xc == 0),
                        stop=(xc == NXC - 1),
                    )
                for h in range(HPC):
                    base = (h * NSK + skc) * 65
                    nc.vector.tensor_copy(
                        vpa_sb[:, base:base + 64],
                        ps[:, h * 64:(h + 1) * 64],
                    )

        ets = {}  # (h, g, jj) -> ET tile awaiting its AV matmuls

        def z_exp(g, jj):
            """Per head: one [128,1024] z tile (skc pair 2jj,2jj+1) + exp.

            exp(z/8 - 4): the -4 bias keeps the scores in fp16 range
            (max z/8 ~ 11.9 -> e^7.9 = 2.7e3) and cancels in the normalize.
            """
            for h in range(HPC):
                z_ps = ps_z.tile([128, 2 * SQW], F32, tag="z",
                                 name=f"z_{h}_{g}_{jj}")
                for half in range(2):
                    skc = 2 * jj + half
                    nc.tensor.matmul(
                        z_ps[:, half * SQW:(half + 1) * SQW],
                        kpt_sb[h * 64:(h + 1) * 64, skc * SKW:(skc + 1) * SKW],
                        qpt_sb[h * 64:(h + 1) * 64, g * SQW:(g + 1) * SQW],
                        start=True,
                        stop=True,
                    )
                et = etp.tile([128, 2 * SQW], F16, tag="et",
                              name=f"et_{h}_{g}_{jj}")
                nc.scalar.activation(et[:], z_ps[:], EXP,
                                     scale=1.0 / 8.0, bias=bias_sb[:])
                ets[(h, g, jj)] = et

        # PSUM accumulation groups must be contiguous per bank (interleaved
        # start/stop groups at different offsets in one bank corrupt the
        # result), so AV runs as per-(h, sq-128-subtile) bursts of 16
        # back-to-back matmuls, one bank each, after the group's exps.
        def av_run(g, h, m):
            acc = ps_av.tile([128, 65], F32, tag="av", name=f"av_{g}_{h}_{m}")
            for jj in range(NJJ):
                et = ets[(h, g, jj)]
                for half in range(2):
                    skc = 2 * jj + half
                    vb = (h * NSK + skc) * 65
                    nc.tensor.matmul(
                        acc[:],
                        et[:, half * SQW + m * 128:half * SQW + (m + 1) * 128],
                        vpa_sb[:, vb:vb + 65],
                        start=(skc == 0),
                        stop=(skc == NSK - 1),
                    )
            return acc

        def unit(g, m, tail=False):
            """One sq-128 tile end to end: 2 AV runs -> normalize ->
            transpose -> output projection -> DMA.

            Mid-stream (ACT busy with exps) everything non-PE runs on DVE;
            in the tail (ACT idle) the muls/copies shift to ACT so the
            serial PE<->DVE chain shortens.
            """
            t = g * 4 + m
            accs = [av_run(g, h, m) for h in range(HPC)]
            hsq = hsqp.tile([128, HD], F16, tag="hsq", name=f"hsq_{t}")
            for h in range(HPC):
                rec = smalls.tile([128, 1], F32, tag="rec", name=f"rec_{t}_{h}")
                nc.vector.reciprocal(rec[:], accs[h][:, 64:65])
                dsth = hsq[:, h * 64:(h + 1) * 64]
                if tail:
                    nc.scalar.mul(dsth, accs[h][:, 0:64], rec[:])
                else:
                    nc.vector.tensor_scalar_mul(dsth, accs[h][:, 0:64], rec[:])
            tr = ps_pr.tile([128, SQW], F16, tag="pr", name=f"tr_{t}")
            nc.tensor.transpose(tr[:, 0:128], hsq[:], ident_sb[:])
            hdst = headst_sb[:, t * 128:(t + 1) * 128]
            if tail:
                nc.scalar.copy(hdst, tr[:, 0:128])
            else:
                nc.vector.tensor_copy(hdst, tr[:, 0:128])
            ot = outp.tile([128, D_M], F16, tag="ot", name=f"ot_{t}")
            for dmc in range(D_M // SQW):
                op = ps_pr.tile([128, SQW], F32, tag="pr", name=f"op_{t}_{dmc}")
                nc.tensor.matmul(
                    op[:],
                    headst_sb[:, t * 128:(t + 1) * 128],
                    wot_sb[:, dmc * SQW:(dmc + 1) * SQW],
                    start=True,
                    stop=True,
                )
                dst = ot[:, dmc * SQW:(dmc + 1) * SQW]
                if tail and dmc % 2:
                    nc.scalar.copy(dst, op[:])
                else:
                    nc.vector.tensor_copy(dst, op[:])
            nc.sync.dma_start(out[t * 128:(t + 1) * 128, :], ot[:])

        # ---- DMA stream (SP queue, in order) ----
        # K0 first; wk lands during its transfer; Q0 is the binding
        # constraint for the first z either way
        tk, tq, tv = {}, {}, {}
        tk[0] = load_chunk(kt, 0, "kc_0")
        load_w(wk, wk_sb)
        tq[0] = load_chunk(qt, 0, "qc_0")
        load_w(wq, wq_sb)
        nc.sync.dma_start(ident_sb[:], ident)
        tk[1] = load_chunk(kt, 1, "kc_1")
        tq[1] = load_chunk(qt, 1, "qc_1")
        load_w(wv, wv_sb)
        tv[0] = load_chunk(vt, 0, "vc_0")
        nc.sync.dma_start(wot_sb[:], wot)
        tk[2] = load_chunk(kt, 2, "kc_2")
        tq[2] = load_chunk(qt, 2, "qc_2")
        tv[1] = load_chunk(vt, 1, "vc_1")
        tk[3] = load_chunk(kt, 3, "kc_3")
        tq[3] = load_chunk(qt, 3, "qc_3")
        tv[2] = load_chunk(vt, 2, "vc_2")
        tv[3] = load_chunk(vt, 3, "vc_3")

        # ones column of VpAug via gpsimd memset (no DMA needed)
        nc.gpsimd.memset(
            vpa_sb[:].rearrange("p (c f) -> p c f", f=65)[:, :, 64:65], 1.0)

        # exp bias constant (-4) as a per-partition scalar AP
        bias_sb = wpool.tile([128, 1], F32, tag="bias")
        nc.gpsimd.memset(bias_sb[:], -4.0)

        # absorb the 1.3us exp table load inside the initial DMA window
        warm = smalls.tile([128, 1], F32, tag="warm")
        nc.scalar.activation(warm[:], bias_sb[:], EXP, scale=1.0)

        # burn the PE pstate ramp (low/mid clock for the first ~3us of a busy
        # stretch) on junk matmuls over memset data, starting ~0.5us in --
        # no DMA dependency, so the first projections run at full clock
        jsrc = wpool.tile([128, 128], F16, tag="jsrc")
        nc.gpsimd.memset(jsrc[:], 0.0)
        junk = ps_pr.tile([128, SQW], F32, tag="pr", name="junk")
        for _ in range(40):
            nc.tensor.matmul(junk[:, 0:128], jsrc[:], jsrc[:],
                             start=True, stop=True)

        def _g3_chain(g3, m):
            t = g3[m]["t"]
            tr = ps_pr.tile([128, SQW], F16, tag="pr", name=f"tr_{t}")
            nc.tensor.transpose(tr[:, 0:128], g3[m]["hsq"][:], ident_sb[:])
            hdst = headst_sb[:, t * 128:(t + 1) * 128]
            nc.scalar.copy(hdst, tr[:, 0:128])
            ot = outp.tile([128, D_M], F16, tag="ot", name=f"ot_{t}")
            for dmc in range(D_M // SQW):
                op = ps_pr.tile([128, SQW], F32, tag="pr", name=f"op_{t}_{dmc}")
                nc.tensor.matmul(
                    op[:],
                    headst_sb[:, t * 128:(t + 1) * 128],
                    wot_sb[:, dmc * SQW:(dmc + 1) * SQW],
                    start=True,
                    stop=True,
                )
                dst = ot[:, dmc * SQW:(dmc + 1) * SQW]
                if dmc % 2:
                    nc.scalar.copy(dst, op[:])
                else:
                    nc.vector.tensor_copy(dst, op[:])
                nc.sync.dma_start(
                    out[t * 128:(t + 1) * 128, dmc * SQW:(dmc + 1) * SQW], dst)

        # ---- compute stream ----
        # c=0: only K0 x Q0 feasible (4 ET tiles)
        project(tk[0], wk_sb, kpt_sb, 0, "pk0")
        project(tq[0], wq_sb, qpt_sb, 0, "pq0")
        for jj in (0, 1):
            z_exp(0, jj)
        # c=1
        project(tk[1], wk_sb, kpt_sb, 1, "pk1")
        project(tq[1], wq_sb, qpt_sb, 1, "pq1")
        for (g, jj) in ((0, 2), (0, 3), (1, 0), (1, 1), (1, 2), (1, 3)):
            z_exp(g, jj)
        project_v(tv[0], 0)
        # c=2
        project(tk[2], wk_sb, kpt_sb, 2, "pk2")
        project(tq[2], wq_sb, qpt_sb, 2, "pq2")
        for (g, jj) in ((0, 4), (0, 5), (1, 4), (1, 5),
                        (2, 0), (2, 1), (2, 2), (2, 3), (2, 4), (2, 5)):
            z_exp(g, jj)
        project_v(tv[1], 1)
        # c=3
        project(tk[3], wk_sb, kpt_sb, 3, "pk3")
        project(tq[3], wq_sb, qpt_sb, 3, "pq3")
        z_exp(0, 6)
        z_exp(0, 7)
        z_exp(1, 6)
        z_exp(1, 7)
        project_v(tv[2], 2)
        z_exp(2, 6)
        z_exp(2, 7)
        project_v(tv[3], 3)
        # group 3's z/exp stream with groups 0-2's finish units interleaved
        # (the units' AV runs become runnable as those groups' last exps land)
        units = [(g, m) for g in (0, 1, 2) for m in range(4)]
        ui = 0
        for jj in range(NJJ):
            z_exp(3, jj)
            for _ in range(2 if 2 <= jj < 6 else 1):
                if ui < len(units):
                    unit(*units[ui])
                    ui += 1
        while ui < len(units):
            unit(*units[ui])
            ui += 1
        # group 3 runs stage-pipelined: by the time its last exp lands only
        # the final 2 AV matmuls of each run remain, so the per-unit work is
        # almost pure cross-engine chain latency -- overlap the stages of
        # consecutive units instead of running each unit end-to-end.
        g3 = {}
        for m in range(4):
            t = 12 + m
            g3[m] = dict(t=t)
            # PE: AV runs (slot for m waits muls of m-1, emitted below)
            g3[m]["accs"] = [av_run(3, h, m) for h in range(HPC)]
            # DVE: recips; ACT: muls into hsq
            hsq = hsqp.tile([128, HD], F16, tag="hsq", name=f"hsq_{t}")
            g3[m]["hsq"] = hsq
            for h in range(HPC):
                rec = smalls.tile([128, 1], F32, tag="rec", name=f"rec_{t}_{h}")
                nc.vector.reciprocal(rec[:], g3[m]["accs"][h][:, 64:65])
                nc.scalar.mul(hsq[:, h * 64:(h + 1) * 64],
                              g3[m]["accs"][h][:, 0:64], rec[:])
            # previous unit's transpose/outproj interleave here so its ACT/DVE
            # stages overlap this unit's AV+normalize
            if m > 0:
                _g3_chain(g3, m - 1)
        _g3_chain(g3, 3)
def _build_nc():
    nc = bacc.Bacc("TRN2", target_bir_lowering=False, debug=False,
                   num_devices=NCORES)
    aps = {
        "qt": nc.dram_tensor("qt", [D_X, S], F16, kind="ExternalInput").ap(),
        "kt": nc.dram_tensor("kt", [D_X, S], F16, kind="ExternalInput").ap(),
        "vt": nc.dram_tensor("vt", [D_X, S], F16, kind="ExternalInput").ap(),
        "wq": nc.dram_tensor("wq", [128, D_X], F16, kind="ExternalInput").ap(),
        "wk": nc.dram_tensor("wk", [128, D_X], F16, kind="ExternalInput").ap(),
        "wv": nc.dram_tensor("wv", [128, D_X], F16, kind="ExternalInput").ap(),
        "wot": nc.dram_tensor("wot", [HD, D_M], F16, kind="ExternalInput").ap(),
        "ident": nc.dram_tensor("ident", [128, 128], F16, kind="ExternalInput").ap(),
        "out": nc.dram_tensor("out", [S, D_M], F16, kind="ExternalOutput").ap(),
    }
    with tile.TileContext(nc) as tc:
        with nc.allow_low_precision(reason="fp16 matmul/softmax pipeline"):
            _emit(tc, nc, aps)
    nc.compile()
    return nc


def kernel(**inputs):
    global LAST_EXEC_NS, _NC_CACHE
    Q = np.asarray(inputs["Q"], dtype=np.float32)
    K = np.asarray(inputs["K"], dtype=np.float32)
    V = np.asarray(inputs["V"], dtype=np.float32)
    W_q = np.asarray(inputs["W_q"], dtype=np.float32)
    W_k = np.asarray(inputs["W_k"], dtype=np.float32)
    W_v = np.asarray(inputs["W_v"], dtype=np.float32)
    W_o = np.asarray(inputs["W_o"], dtype=np.float32)

    def _pack_w(W, h0):
        # device SBUF layout [p, c*128+f] = W[c*128+p, f]; full-rate DMA rows
        w = np.concatenate([W[h0 + i] for i in range(HPC)], axis=1)  # (D_X, HD)
        return np.ascontiguousarray(
            w.reshape(NXC, 128, HD).transpose(1, 0, 2).reshape(128, NXC * HD)
            .astype(np.float16))

    QT = np.ascontiguousarray(Q.T.astype(np.float16))
    KT = np.ascontiguousarray(K.T.astype(np.float16))
    VT = np.ascontiguousarray(V.T.astype(np.float16))
    ident = np.eye(128, dtype=np.float16)
    in_maps = []
    for c in range(NCORES):
        h0 = HPC * c
        in_maps.append({
            "qt": QT, "kt": KT, "vt": VT,
            "wq": _pack_w(W_q, h0), "wk": _pack_w(W_k, h0),
            "wv": _pack_w(W_v, h0),
            "wot": np.ascontiguousarray(
                W_o[:, c * HD:(c + 1) * HD].T.astype(np.float16)),
            "ident": ident,
        })

    if _NC_CACHE is None:
        _NC_CACHE = _build_nc()
    nc = _NC_CACHE

    trace = bool(os.environ.get("MHA_TRACE"))
    res = None
    if trace:
        try:
            res = run_bass_kernel_spmd(nc, in_maps, list(range(NCORES)),
                                       trace=True)
        except Exception as e:  # profiling infra unavailable -> run untraced
            print(f"[kernel] traced run failed ({e!r}); falling back")
            res = None
    if res is None:
        res = run_bass_kernel_spmd(nc, in_maps, list(range(NCORES)))

    LAST_EXEC_NS = getattr(res, "exec_time_ns", None)

    out = np.zeros((S, D_M), np.float32)
    for r in res.results:
        out += r["out"].astype(np.float32)
    return out


# revision 12
# speedup vs baseline: 1.0013x; 1.0013x over previous
"""Multi-head attention (16 heads, S=2048, d_model=1024, d_head=64) on 8 TRN2
NeuronCores, tensor-parallel over heads (2 heads per core).

Restructured from the 120us baseline around the TimelineSim cost model, where
matmul cost = output-free-size rows (K and M are free):

  * AV matmuls run transposed: out[sq=128, dv+1=65] with the exp tile as the
    stationary operand, 65 rows per accumulation step instead of 512 -> PE
    drops from 196k to 166k rows (~82us -> ~69us busy).
  * softmax normalize becomes a per-partition tensor_scalar (denominator is
    column 64 of the accumulator) -- no gpsimd broadcast.
  * z PSUM tiles are [128, 1024] (2 banks); exp runs 64x instead of 128x,
    halving the fixed per-instruction ACT overhead (~81us -> ~66us busy).
  * everything 2-byte: fp16 in/out of every matmul, exp computes
    exp(z/8 - 4) so the scores fit fp16 (max z/8 ~ 11.9), the bias cancels
    in the normalize. Output ships fp16 (half the out DMA), host sums in f32.
  * heads come out of AV as [sq, dv]; a 16x[128,128] PE transpose (+DVE copy)
    restores [hd, sq] for the output projection.

Schedule: inputs stream K0,Q0,K1,Q1,V0,K2,Q2,V1,K3,Q3,V2,V3 so the exp
stream (the ~66us ACT critical path, the pacer) starts by ~7us and never
starves. z/exp tiles are emitted greedily (group-ascending); group 0's AV
rides inline behind the V projections; groups 1-3 accumulate at the end of
the stream in group order, each followed by its normalize/transpose/outproj/
DMA chain so only group 3's chain is a tail. PSUM: 2x[128,1024] z +
2x[128,260] AV accumulators + 2x[128,512] proj/outproj = exactly 8 banks.
"""

import os

import numpy as np

import concourse.bass as bass
import concourse.tile as tile
from concourse import bacc, mybir
from concourse.bass_utils import run_bass_kernel_spmd

HEADS, D_K, D_V, D_X, D_M, S = 16, 64, 64, 1024, 1024, 2048
NCORES = 8
HPC = HEADS // NCORES          # heads per core
HD = HPC * D_K                 # 128: stacked head dim per core
SQW = 512                      # sq group width
NSQ = S // SQW                 # 4 groups
SKW = 128                      # sk chunk width (partition dim)
NSK = S // SKW                 # 16
NXC = D_X // 128               # 8 contraction chunks for projections
NJJ = NSK // 2                 # 8 skc-pairs (one [128,1024] z tile each)

F32 = mybir.dt.float32
F16 = mybir.dt.float16
EXP = mybir.ActivationFunctionType.Exp

LAST_EXEC_NS = None
_NC_CACHE = None


def _emit(tc, nc, aps):
    from contextlib import ExitStack

    qt, kt, vt, wq, wk, wv, wot, ident, out = (
        aps["qt"], aps["kt"], aps["vt"], aps["wq"], aps["wk"], aps["wv"],
        aps["wot"], aps["ident"], aps["out"],
    )

    with ExitStack() as ctx:
        wpool = ctx.enter_context(tc.tile_pool(name="weights", bufs=1))
        proj = ctx.enter_context(tc.tile_pool(name="proj", bufs=1))
        inp = ctx.enter_context(tc.tile_pool(name="inp", bufs=6))
        etp = ctx.enter_context(tc.tile_pool(name="et", bufs=52))
        hsqp = ctx.enter_context(tc.tile_pool(name="hsq", bufs=8))
        outp = ctx.enter_context(tc.tile_pool(name="outs", bufs=4))
        smalls = ctx.enter_context(tc.tile_pool(name="smalls", bufs=4))
        ps_z = ctx.enter_context(tc.tile_pool(name="ps_z", bufs=2, space="PSUM"))
        ps_av = ctx.enter_context(tc.tile_pool(name="ps_av", bufs=2, space="PSUM"))
        ps_pr = ctx.enter_context(tc.tile_pool(name="ps_pr", bufs=2, space="PSUM"))

        # ---- persistent SBUF tensors ----
        wq_sb = wpool.tile([128, D_X], F16, tag="wq")     # (xc p) stacked chunks
        wk_sb = wpool.tile([128, D_X], F16, tag="wk")
        wv_sb = wpool.tile([128, D_X], F16, tag="wv")
        wot_sb = wpool.tile([HD, D_M], F16, tag="wot")
        ident_sb = wpool.tile([128, 128], F16, tag="ident")
        qpt_sb = proj.tile([HD, S], F16, tag="qpt")
        kpt_sb = proj.tile([HD, S], F16, tag="kpt")
        # VpAug: per (h, skc) a (128 sk, 65) block: cols 0-63 = Vp, col 64 = 1
        vpa_sb = proj.tile([128, HPC * NSK * 65], F16, tag="vpa")
        headst_sb = proj.tile([HD, S], F16, tag="headst")

        def load_w(w_dram, w_sb):
            nc.sync.dma_start(w_sb[:], w_dram)

        def load_chunk(tt_dram, c, name, lo=0, w=SQW, tag="inp"):
            """One DMA: all 8 xc strips of cols [c*512+lo, +w) -> (128, 8, w)."""
            t = inp.tile([128, NXC, w], F16, tag=tag, name=name,
                         bufs=2 if tag == "inp0" else None)
            nc.sync.dma_start(
                t[:],
                tt_dram.rearrange("(xc p) s -> p xc s", p=128)[
                    :, :, c * SQW + lo:c * SQW + lo + w
                ],
            )
            return t

        def project(t, w_sb, dst_sb, c, name, lo=0, w=SQW):
            """dst_sb[:, c*512+lo : +w] = W.T @ X.T chunk cols (fp16)."""
            ps = ps_pr.tile([128, w], F32, tag="pr", name=name,
                            padded_shape=[128, SQW])
            for xc in range(NXC):
                nc.tensor.matmul(
                    ps[:],
                    w_sb[:, xc * 128:(xc + 1) * 128],
                    t[:, xc, :],
                    start=(xc == 0),
                    stop=(xc == NXC - 1),
                )
            nc.vector.tensor_copy(
                dst_sb[:, c * SQW + lo:c * SQW + lo + w], ps[:])

        def project_v(t, c):
            """VpAug sk-chunks for 512-chunk c: Vp = VT_chunk.T @ Wv directly
            in (sk, hd) layout."""
            for j in range(SQW // SKW):
                skc = c * (SQW // SKW) + j
                ps = ps_pr.tile([128, HD], F32, tag="pr", name=f"vp_{skc}",
                                padded_shape=[128, SQW])
                for xc in range(NXC):
                    nc.tensor.matmul(
                        ps[:],
                        t[:, xc, j * SKW:(j + 1) * SKW],
                        wv_sb[:, xc * 128:(xc + 1) * 128],
                        start=(xc == 0),
                        stop=(xc == NXC - 1),
                    )
                for h in range(HPC):
                    base = (h * NSK + skc) * 65
                    nc.vector.tensor_copy(
                        vpa_sb[:, base:base + 64],
                        ps[:, h * 64:(h + 1) * 64],
                    )

        ets = {}  # (h, g, jj) -> ET tile awaiting its AV matmuls

        def z_exp(g, jj):
            """Per head: one [128,1024] z tile (skc pair 2jj,2jj+1) + exp.

            exp(z/8 - 4): the -4 bias keeps the scores in fp16 range
            (max z/8 ~ 11.9 -> e^7.9 = 2.7e3) and cancels in the normalize.
            """
            for h in range(HPC):
                z_ps = ps_z.tile([128, 2 * SQW], F32, tag="z",
                                 name=f"z_{h}_{g}_{jj}")
                for half in range(2):
                    skc = 2 * jj + half
                    nc.tensor.matmul(
                        z_ps[:, half * SQW:(half + 1) * SQW],
                        kpt_sb[h * 64:(h + 1) * 64, skc * SKW:(skc + 1) * SKW],
                        qpt_sb[h * 64:(h + 1) * 64, g * SQW:(g + 1) * SQW],
                        start=True,
                        stop=True,
                    )
                et = etp.tile([128, 2 * SQW], F16, tag="et",
                              name=f"et_{h}_{g}_{jj}")
                nc.scalar.activation(et[:], z_ps[:], EXP,
                                     scale=1.0 / 8.0, bias=bias_sb[:])
                ets[(h, g, jj)] = et

        # PSUM accumulation groups must be contiguous per bank (interleaved
        # start/stop groups at different offsets in one bank corrupt the
        # result), so AV runs as per-(h, sq-128-subtile) bursts of 16
        # back-to-back matmuls, one bank each, after the group's exps.
        def av_run(g, h, m):
            acc = ps_av.tile([128, 65], F32, tag="av", name=f"av_{g}_{h}_{m}")
            for jj in range(NJJ):
                et = ets[(h, g, jj)]
                for half in range(2):
                    skc = 2 * jj + half
                    vb = (h * NSK + skc) * 65
                    nc.tensor.matmul(
                        acc[:],
                        et[:, half * SQW + m * 128:half * SQW + (m + 1) * 128],
                        vpa_sb[:, vb:vb + 65],
                        start=(skc == 0),
                        stop=(skc == NSK - 1),
                    )
            return acc

        def unit(g, m, tail=False):
            """One sq-128 tile end to end: 2 AV runs -> normalize ->
            transpose -> output projection -> DMA.

            Mid-stream (ACT busy with exps) everything non-PE runs on DVE;
            in the tail (ACT idle) the muls/copies shift to ACT so the
            serial PE<->DVE chain shortens.
            """
            t = g * 4 + m
            accs = [av_run(g, h, m) for h in range(HPC)]
            hsq = hsqp.tile([128, HD], F16, tag="hsq", name=f"hsq_{t}")
            for h in range(HPC):
                rec = smalls.tile([128, 1], F32, tag="rec", name=f"rec_{t}_{h}")
                nc.vector.reciprocal(rec[:], accs[h][:, 64:65])
                dsth = hsq[:, h * 64:(h + 1) * 64]
                if tail:
                    nc.scalar.mul(dsth, accs[h][:, 0:64], rec[:])
                else:
                    nc.vector.tensor_scalar_mul(dsth, accs[h][:, 0:64], rec[:])
            tr = ps_pr.tile([128, SQW], F16, tag="pr", name=f"tr_{t}")
            nc.tensor.transpose(tr[:, 0:128], hsq[:], ident_sb[:])
            hdst = headst_sb[:, t * 128:(t + 1) * 128]
            if tail:
                nc.scalar.copy(hdst, tr[:, 0:128])
            else:
                nc.vector.tensor_copy(hdst, tr[:, 0:128])
            ot = outp.tile([128, D_M], F16, tag="ot", name=f"ot_{t}")
            for dmc in range(D_M // SQW):
                op = ps_pr.tile([128, SQW], F32, tag="pr", name=f"op_{t}_{dmc}")
                nc.tensor.matmul(
                    op[:],
                    headst_sb[:, t * 128:(t + 1) * 128],
                    wot_sb[:, dmc * SQW:(dmc + 1) * SQW],
                    start=True,
                    stop=True,
                )
                dst = ot[:, dmc * SQW:(dmc + 1) * SQW]
                if tail and dmc % 2:
                    nc.scalar.copy(dst, op[:])
                else:
                    nc.vector.tensor_copy(dst, op[:])
            nc.sync.dma_start(out[t * 128:(t + 1) * 128, :], ot[:])

        # ---- DMA stream (SP queue, in order) ----
        # wk/wq first (small, full-rate), then Q0 and K0 in 256-halves: the
        # first z tile only needs kpt cols 0:256, so the exp stream starts
        # ~3.5us earlier than with a monolithic K0
        tk, tq, tv = {}, {}, {}
        load_w(wk, wk_sb)
        load_w(wq, wq_sb)
        tq[0] = load_chunk(qt, 0, "qc_0")
        k0a = load_chunk(kt, 0, "kc_0a", lo=0, w=256, tag="inp0")
        k0b = load_chunk(kt, 0, "kc_0b", lo=256, w=256, tag="inp0")
        nc.sync.dma_start(ident_sb[:], ident)
        tk[1] = load_chunk(kt, 1, "kc_1")
        tq[1] = load_chunk(qt, 1, "qc_1")
        load_w(wv, wv_sb)
        tv[0] = load_chunk(vt, 0, "vc_0")
        nc.sync.dma_start(wot_sb[:], wot)
        tk[2] = load_chunk(kt, 2, "kc_2")
        tq[2] = load_chunk(qt, 2, "qc_2")
        tv[1] = load_chunk(vt, 1, "vc_1")
        tk[3] = load_chunk(kt, 3, "kc_3")
        tq[3] = load_chunk(qt, 3, "qc_3")
        tv[2] = load_chunk(vt, 2, "vc_2")
        tv[3] = load_chunk(vt, 3, "vc_3")

        # ones column of VpAug via gpsimd memset (no DMA needed)
        nc.gpsimd.memset(
            vpa_sb[:].rearrange("p (c f) -> p c f", f=65)[:, :, 64:65], 1.0)

        # exp bias constant (-4) as a per-partition scalar AP
        bias_sb = wpool.tile([128, 1], F32, tag="bias")
        nc.gpsimd.memset(bias_sb[:], -4.0)

        # absorb the 1.3us exp table load inside the initial DMA window
        warm = smalls.tile([128, 1], F32, tag="warm")
        nc.scalar.activation(warm[:], bias_sb[:], EXP, scale=1.0)

        # burn the PE pstate ramp (low/mid clock for the first ~3us of a busy
        # stretch) on junk matmuls over memset data, starting ~0.5us in --
        # no DMA dependency, so the first projections run at full clock
        jsrc = wpool.tile([128, 128], F16, tag="jsrc")
        nc.gpsimd.memset(jsrc[:], 0.0)
        junk = ps_pr.tile([128, SQW], F32, tag="pr", name="junk")
        for _ in range(30):
            nc.tensor.matmul(junk[:, 0:128], jsrc[:], jsrc[:],
                             start=True, stop=True)

        def _g3_chain(g3, m):
            t = g3[m]["t"]
            tr = ps_pr.tile([128, SQW], F16, tag="pr", name=f"tr_{t}")
            nc.tensor.transpose(tr[:, 0:128], g3[m]["hsq"][:], ident_sb[:])
            hdst = headst_sb[:, t * 128:(t + 1) * 128]
            nc.scalar.copy(hdst, tr[:, 0:128])
            ot = outp.tile([128, D_M], F16, tag="ot", name=f"ot_{t}")
            for dmc in range(D_M // SQW):
                op = ps_pr.tile([128, SQW], F32, tag="pr", name=f"op_{t}_{dmc}")
                nc.tensor.matmul(
                    op[:],
                    headst_sb[:, t * 128:(t + 1) * 128],
                    wot_sb[:, dmc * SQW:(dmc + 1) * SQW],
                    start=True,
                    stop=True,
                )
                dst = ot[:, dmc * SQW:(dmc + 1) * SQW]
                if dmc % 2:
                    nc.scalar.copy(dst, op[:])
                else:
                    nc.vector.tensor_copy(dst, op[:])
                nc.sync.dma_start(
                    out[t * 128:(t + 1) * 128, dmc * SQW:(dmc + 1) * SQW], dst)

        # ---- compute stream ----
        # c=0: only K0 x Q0 feasible (4 ET tiles)
        project(tq[0], wq_sb, qpt_sb, 0, "pq0")
        project(k0a, wk_sb, kpt_sb, 0, "pk0a", lo=0, w=256)
        z_exp(0, 0)
        project(k0b, wk_sb, kpt_sb, 0, "pk0b", lo=256, w=256)
        z_exp(0, 1)
        # c=1
        project(tk[1], wk_sb, kpt_sb, 1, "pk1")
        project(tq[1], wq_sb, qpt_sb, 1, "pq1")
        for (g, jj) in ((0, 2), (0, 3), (1, 0), (1, 1), (1, 2), (1, 3)):
            z_exp(g, jj)
        project_v(tv[0], 0)
        # c=2
        project(tk[2], wk_sb, kpt_sb, 2, "pk2")
        project(tq[2], wq_sb, qpt_sb, 2, "pq2")
        for (g, jj) in ((0, 4), (0, 5), (1, 4), (1, 5),
                        (2, 0), (2, 1), (2, 2), (2, 3), (2, 4), (2, 5)):
            z_exp(g, jj)
        project_v(tv[1], 1)
        # c=3
        project(tk[3], wk_sb, kpt_sb, 3, "pk3")
        project(tq[3], wq_sb, qpt_sb, 3, "pq3")
        z_exp(0, 6)
        z_exp(0, 7)
        z_exp(1, 6)
        z_exp(1, 7)
        project_v(tv[2], 2)
        z_exp(2, 6)
        z_exp(2, 7)
        project_v(tv[3], 3)
        # group 3's z/exp stream with groups 0-2's finish units interleaved
        # (the units' AV runs become runnable as those groups' last exps land)
        units = [(g, m) for g in (0, 1, 2) for m in range(4)]
        ui = 0
        for jj in range(NJJ):
            z_exp(3, jj)
            for _ in range(2 if 2 <= jj < 6 else 1):
                if ui < len(units):
                    unit(*units[ui])
                    ui += 1
        while ui < len(units):
            unit(*units[ui])
            ui += 1
        # group 3 runs stage-pipelined: by the time its last exp lands only
        # the final 2 AV matmuls of each run remain, so the per-unit work is
        # almost pure cross-engine chain latency -- overlap the stages of
        # consecutive units instead of running each unit end-to-end.
        g3 = {}
        for m in range(4):
            t = 12 + m
            g3[m] = dict(t=t)
            # PE: AV runs (slot for m waits muls of m-1, emitted below)
            g3[m]["accs"] = [av_run(3, h, m) for h in range(HPC)]
            # DVE: recips; ACT: muls into hsq
            hsq = hsqp.tile([128, HD], F16, tag="hsq", name=f"hsq_{t}")
            g3[m]["hsq"] = hsq
            for h in range(HPC):
                rec = smalls.tile([128, 1], F32, tag="rec", name=f"rec_{t}_{h}")
                nc.vector.reciprocal(rec[:], g3[m]["accs"][h][:, 64:65])
                nc.scalar.mul(hsq[:, h * 64:(h + 1) * 64],
                              g3[m]["accs"][h][:, 0:64], rec[:])
            # previous unit's transpose/outproj interleave here so its ACT/DVE
            # stages overlap this unit's AV+normalize
            if m > 0:
                _g3_chain(g3, m - 1)
        _g3_chain(g3, 3)
def _build_nc():
    nc = bacc.Bacc("TRN2", target_bir_lowering=False, debug=False,
                   num_devices=NCORES)
    aps = {
        "qt": nc.dram_tensor("qt", [D_X, S], F16, kind="ExternalInput").ap(),
        "kt": nc.dram_tensor("kt", [D_X, S], F16, kind="ExternalInput").ap(),
        "vt": nc.dram_tensor("vt", [D_X, S], F16, kind="ExternalInput").ap(),
        "wq": nc.dram_tensor("wq", [128, D_X], F16, kind="ExternalInput").ap(),
        "wk": nc.dram_tensor("wk", [128, D_X], F16, kind="ExternalInput").ap(),
        "wv": nc.dram_tensor("wv", [128, D_X], F16, kind="ExternalInput").ap(),
        "wot": nc.dram_tensor("wot", [HD, D_M], F16, kind="ExternalInput").ap(),
        "ident": nc.dram_tensor("ident", [128, 128], F16, kind="ExternalInput").ap(),
        "out": nc.dram_tensor("out", [S, D_M], F16, kind="ExternalOutput").ap(),
    }
    with tile.TileContext(nc) as tc:
        with nc.allow_low_precision(reason="fp16 matmul/softmax pipeline"):
            _emit(tc, nc, aps)
    nc.compile()
    return nc


def kernel(**inputs):
    global LAST_EXEC_NS, _NC_CACHE
    Q = np.asarray(inputs["Q"], dtype=np.float32)
    K = np.asarray(inputs["K"], dtype=np.float32)
    V = np.asarray(inputs["V"], dtype=np.float32)
    W_q = np.asarray(inputs["W_q"], dtype=np.float32)
    W_k = np.asarray(inputs["W_k"], dtype=np.float32)
    W_v = np.asarray(inputs["W_v"], dtype=np.float32)
    W_o = np.asarray(inputs["W_o"], dtype=np.float32)

    def _pack_w(W, h0):
        # device SBUF layout [p, c*128+f] = W[c*128+p, f]; full-rate DMA rows
        w = np.concatenate([W[h0 + i] for i in range(HPC)], axis=1)  # (D_X, HD)
        return np.ascontiguousarray(
            w.reshape(NXC, 128, HD).transpose(1, 0, 2).reshape(128, NXC * HD)
            .astype(np.float16))

    QT = np.ascontiguousarray(Q.T.astype(np.float16))
    KT = np.ascontiguousarray(K.T.astype(np.float16))
    VT = np.ascontiguousarray(V.T.astype(np.float16))
    ident = np.eye(128, dtype=np.float16)
    in_maps = []
    for c in range(NCORES):
        h0 = HPC * c
        in_maps.append({
            "qt": QT, "kt": KT, "vt": VT,
            "wq": _pack_w(W_q, h0), "wk": _pack_w(W_k, h0),
            "wv": _pack_w(W_v, h0),
            "wot": np.ascontiguousarray(
                W_o[:, c * HD:(c + 1) * HD].T.astype(np.float16)),
            "ident": ident,
        })

    if _NC_CACHE is None:
        _NC_CACHE = _build_nc()
    nc = _NC_CACHE

    trace = bool(os.environ.get("MHA_TRACE"))
    res = None
    if trace:
        try:
            res = run_bass_kernel_spmd(nc, in_maps, list(range(NCORES)),
                                       trace=True)
        except Exception as e:  # profiling infra unavailable -> run untraced
            print(f"[kernel] traced run failed ({e!r}); falling back")
            res = None
    if res is None:
        res = run_bass_kernel_spmd(nc, in_maps, list(range(NCORES)))

    LAST_EXEC_NS = getattr(res, "exec_time_ns", None)

    out = np.zeros((S, D_M), np.float32)
    for r in res.results:
        out += r["out"].astype(np.float32)
    return out


# revision 13
# speedup vs baseline: 1.0125x; 1.0112x over previous
"""Multi-head attention (16 heads, S=2048, d_model=1024, d_head=64) on 8 TRN2
NeuronCores, tensor-parallel over heads (2 heads per core).

Restructured from the 120us baseline around the TimelineSim cost model, where
matmul cost = output-free-size rows (K and M are free):

  * AV matmuls run transposed: out[sq=128, dv+1=65] with the exp tile as the
    stationary operand, 65 rows per accumulation step instead of 512 -> PE
    drops from 196k to 166k rows (~82us -> ~69us busy).
  * softmax normalize becomes a per-partition tensor_scalar (denominator is
    column 64 of the accumulator) -- no gpsimd broadcast.
  * z PSUM tiles are [128, 1024] (2 banks); exp runs 64x instead of 128x,
    halving the fixed per-instruction ACT overhead (~81us -> ~66us busy).
  * everything 2-byte: fp16 in/out of every matmul, exp computes
    exp(z/8 - 4) so the scores fit fp16 (max z/8 ~ 11.9), the bias cancels
    in the normalize. Output ships fp16 (half the out DMA), host sums in f32.
  * heads come out of AV as [sq, dv]; a 16x[128,128] PE transpose (+DVE copy)
    restores [hd, sq] for the output projection.

Schedule: inputs stream K0,Q0,K1,Q1,V0,K2,Q2,V1,K3,Q3,V2,V3 so the exp
stream (the ~66us ACT critical path, the pacer) starts by ~7us and never
starves. z/exp tiles are emitted greedily (group-ascending); group 0's AV
rides inline behind the V projections; groups 1-3 accumulate at the end of
the stream in group order, each followed by its normalize/transpose/outproj/
DMA chain so only group 3's chain is a tail. PSUM: 2x[128,1024] z +
2x[128,260] AV accumulators + 2x[128,512] proj/outproj = exactly 8 banks.
"""

import os

import numpy as np

import concourse.bass as bass
import concourse.tile as tile
from concourse import bacc, mybir
from concourse.bass_utils import run_bass_kernel_spmd

HEADS, D_K, D_V, D_X, D_M, S = 16, 64, 64, 1024, 1024, 2048
NCORES = 8
HPC = HEADS // NCORES          # heads per core
HD = HPC * D_K                 # 128: stacked head dim per core
SQW = 512                      # sq group width
NSQ = S // SQW                 # 4 groups
SKW = 128                      # sk chunk width (partition dim)
NSK = S // SKW                 # 16
NXC = D_X // 128               # 8 contraction chunks for projections
NJJ = NSK // 2                 # 8 skc-pairs (one [128,1024] z tile each)

F32 = mybir.dt.float32
F16 = mybir.dt.float16
EXP = mybir.ActivationFunctionType.Exp

LAST_EXEC_NS = None
_NC_CACHE = None


def _emit(tc, nc, aps):
    from contextlib import ExitStack

    qt, kt, vt, wq, wk, wv, wot, ident, out = (
        aps["qt"], aps["kt"], aps["vt"], aps["wq"], aps["wk"], aps["wv"],
        aps["wot"], aps["ident"], aps["out"],
    )

    with ExitStack() as ctx:
        wpool = ctx.enter_context(tc.tile_pool(name="weights", bufs=1))
        proj = ctx.enter_context(tc.tile_pool(name="proj", bufs=1))
        inp = ctx.enter_context(tc.tile_pool(name="inp", bufs=6))
        etp = ctx.enter_context(tc.tile_pool(name="et", bufs=52))
        hsqp = ctx.enter_context(tc.tile_pool(name="hsq", bufs=8))
        outp = ctx.enter_context(tc.tile_pool(name="outs", bufs=4))
        smalls = ctx.enter_context(tc.tile_pool(name="smalls", bufs=4))
        ps_z = ctx.enter_context(tc.tile_pool(name="ps_z", bufs=2, space="PSUM"))
        ps_av = ctx.enter_context(tc.tile_pool(name="ps_av", bufs=2, space="PSUM"))
        ps_pr = ctx.enter_context(tc.tile_pool(name="ps_pr", bufs=2, space="PSUM"))

        # ---- persistent SBUF tensors ----
        wq_sb = wpool.tile([128, D_X], F16, tag="wq")     # (xc p) stacked chunks
        wk_sb = wpool.tile([128, D_X], F16, tag="wk")
        wv_sb = wpool.tile([128, D_X], F16, tag="wv")
        wot_sb = wpool.tile([HD, D_M], F16, tag="wot")
        ident_sb = wpool.tile([128, 128], F16, tag="ident")
        qpt_sb = proj.tile([HD, S], F16, tag="qpt")
        kpt_sb = proj.tile([HD, S], F16, tag="kpt")
        # VpAug: per (h, skc) a (128 sk, 65) block: cols 0-63 = Vp, col 64 = 1
        vpa_sb = proj.tile([128, HPC * NSK * 65], F16, tag="vpa")
        headst_sb = proj.tile([HD, S], F16, tag="headst")

        def load_w(w_dram, w_sb):
            nc.sync.dma_start(w_sb[:], w_dram)

        def load_chunk(tt_dram, c, name, lo=0, w=SQW, tag="inp"):
            """One DMA: all 8 xc strips of cols [c*512+lo, +w) -> (128, 8, w)."""
            t = inp.tile([128, NXC, w], F16, tag=tag, name=name,
                         bufs=2 if tag == "inp0" else None)
            nc.sync.dma_start(
                t[:],
                tt_dram.rearrange("(xc p) s -> p xc s", p=128)[
                    :, :, c * SQW + lo:c * SQW + lo + w
                ],
            )
            return t

        def project(t, w_sb, dst_sb, c, name, lo=0, w=SQW):
            """dst_sb[:, c*512+lo : +w] = W.T @ X.T chunk cols (fp16)."""
            ps = ps_pr.tile([128, w], F32, tag="pr", name=name,
                            padded_shape=[128, SQW])
            for xc in range(NXC):
                nc.tensor.matmul(
                    ps[:],
                    w_sb[:, xc * 128:(xc + 1) * 128],
                    t[:, xc, :],
                    start=(xc == 0),
                    stop=(xc == NXC - 1),
                )
            nc.vector.tensor_copy(
                dst_sb[:, c * SQW + lo:c * SQW + lo + w], ps[:])

        def project_v(t, c):
            """VpAug sk-chunks for 512-chunk c: Vp = VT_chunk.T @ Wv directly
            in (sk, hd) layout."""
            for j in range(SQW // SKW):
                skc = c * (SQW // SKW) + j
                ps = ps_pr.tile([128, HD], F32, tag="pr", name=f"vp_{skc}",
                                padded_shape=[128, SQW])
                for xc in range(NXC):
                    nc.tensor.matmul(
                        ps[:],
                        t[:, xc, j * SKW:(j + 1) * SKW],
                        wv_sb[:, xc * 128:(xc + 1) * 128],
                        start=(xc == 0),
                        stop=(xc == NXC - 1),
                    )
                for h in range(HPC):
                    base = (h * NSK + skc) * 65
                    nc.vector.tensor_copy(
                        vpa_sb[:, base:base + 64],
                        ps[:, h * 64:(h + 1) * 64],
                    )

        ets = {}  # (h, g, jj) -> ET tile awaiting its AV matmuls

        def z_exp(g, jj):
            """Per head: one [128,1024] z tile (skc pair 2jj,2jj+1) + exp.

            exp(z/8 - 4): the -4 bias keeps the scores in fp16 range
            (max z/8 ~ 11.9 -> e^7.9 = 2.7e3) and cancels in the normalize.
            """
            for h in range(HPC):
                z_ps = ps_z.tile([128, 2 * SQW], F32, tag="z",
                                 name=f"z_{h}_{g}_{jj}")
                for half in range(2):
                    skc = 2 * jj + half
                    nc.tensor.matmul(
                        z_ps[:, half * SQW:(half + 1) * SQW],
                        kpt_sb[h * 64:(h + 1) * 64, skc * SKW:(skc + 1) * SKW],
                        qpt_sb[h * 64:(h + 1) * 64, g * SQW:(g + 1) * SQW],
                        start=True,
                        stop=True,
                    )
                et = etp.tile([128, 2 * SQW], F16, tag="et",
                              name=f"et_{h}_{g}_{jj}")
                nc.scalar.activation(et[:], z_ps[:], EXP,
                                     scale=1.0 / 8.0, bias=bias_sb[:])
                ets[(h, g, jj)] = et

        # PSUM accumulation groups must be contiguous per bank (interleaved
        # start/stop groups at different offsets in one bank corrupt the
        # result), so AV runs as per-(h, sq-128-subtile) bursts of 16
        # back-to-back matmuls, one bank each, after the group's exps.
        def av_run(g, h, m):
            acc = ps_av.tile([128, 65], F32, tag="av", name=f"av_{g}_{h}_{m}")
            for jj in range(NJJ):
                et = ets[(h, g, jj)]
                for half in range(2):
                    skc = 2 * jj + half
                    vb = (h * NSK + skc) * 65
                    nc.tensor.matmul(
                        acc[:],
                        et[:, half * SQW + m * 128:half * SQW + (m + 1) * 128],
                        vpa_sb[:, vb:vb + 65],
                        start=(skc == 0),
                        stop=(skc == NSK - 1),
                    )
            return acc

        def unit(g, m, tail=False):
            """One sq-128 tile end to end: 2 AV runs -> normalize ->
            transpose -> output projection -> DMA.

            Mid-stream (ACT busy with exps) everything non-PE runs on DVE;
            in the tail (ACT idle) the muls/copies shift to ACT so the
            serial PE<->DVE chain shortens.
            """
            t = g * 4 + m
            accs = [av_run(g, h, m) for h in range(HPC)]
            hsq = hsqp.tile([128, HD], F16, tag="hsq", name=f"hsq_{t}")
            for h in range(HPC):
                rec = smalls.tile([128, 1], F32, tag="rec", name=f"rec_{t}_{h}")
                nc.vector.reciprocal(rec[:], accs[h][:, 64:65])
                dsth = hsq[:, h * 64:(h + 1) * 64]
                if tail:
                    nc.scalar.mul(dsth, accs[h][:, 0:64], rec[:])
                else:
                    nc.vector.tensor_scalar_mul(dsth, accs[h][:, 0:64], rec[:])
            tr = ps_pr.tile([128, SQW], F16, tag="pr", name=f"tr_{t}")
            nc.tensor.transpose(tr[:, 0:128], hsq[:], ident_sb[:])
            hdst = headst_sb[:, t * 128:(t + 1) * 128]
            if tail:
                nc.scalar.copy(hdst, tr[:, 0:128])
            else:
                nc.vector.tensor_copy(hdst, tr[:, 0:128])
            ot = outp.tile([128, D_M], F16, tag="ot", name=f"ot_{t}")
            for dmc in range(D_M // SQW):
                op = ps_pr.tile([128, SQW], F32, tag="pr", name=f"op_{t}_{dmc}")
                nc.tensor.matmul(
                    op[:],
                    headst_sb[:, t * 128:(t + 1) * 128],
                    wot_sb[:, dmc * SQW:(dmc + 1) * SQW],
                    start=True,
                    stop=True,
                )
                dst = ot[:, dmc * SQW:(dmc + 1) * SQW]
                if tail and dmc % 2:
                    nc.scalar.copy(dst, op[:])
                else:
                    nc.vector.tensor_copy(dst, op[:])
            nc.sync.dma_start(out[t * 128:(t + 1) * 128, :], ot[:])

        # ---- DMA stream (SP queue, in order) ----
        # wk/wq first (small, full-rate), then Q0 and K0 in 256-halves: the
        # first z tile only needs kpt cols 0:256, so the exp stream starts
        # ~3.5us earlier than with a monolithic K0
        tk, tq, tv = {}, {}, {}
        load_w(wk, wk_sb)
        load_w(wq, wq_sb)
        tq[0] = load_chunk(qt, 0, "qc_0")
        k0a = load_chunk(kt, 0, "kc_0a", lo=0, w=256, tag="inp0")
        k0b = load_chunk(kt, 0, "kc_0b", lo=256, w=256, tag="inp0")
        nc.sync.dma_start(ident_sb[:], ident)
        tk[1] = load_chunk(kt, 1, "kc_1")
        tq[1] = load_chunk(qt, 1, "qc_1")
        load_w(wv, wv_sb)
        tv[0] = load_chunk(vt, 0, "vc_0")
        nc.sync.dma_start(wot_sb[:], wot)
        tk[2] = load_chunk(kt, 2, "kc_2")
        tq[2] = load_chunk(qt, 2, "qc_2")
        tv[1] = load_chunk(vt, 1, "vc_1")
        tk[3] = load_chunk(kt, 3, "kc_3")
        tq[3] = load_chunk(qt, 3, "qc_3")
        tv[2] = load_chunk(vt, 2, "vc_2")
        tv[3] = load_chunk(vt, 3, "vc_3")

        # ones column of VpAug via gpsimd memset (no DMA needed)
        nc.gpsimd.memset(
            vpa_sb[:].rearrange("p (c f) -> p c f", f=65)[:, :, 64:65], 1.0)

        # exp bias constant (-4) as a per-partition scalar AP
        bias_sb = wpool.tile([128, 1], F32, tag="bias")
        nc.gpsimd.memset(bias_sb[:], -4.0)

        # absorb the 1.3us exp table load inside the initial DMA window
        warm = smalls.tile([128, 1], F32, tag="warm")
        nc.scalar.activation(warm[:], bias_sb[:], EXP, scale=1.0)

        # burn the PE pstate ramp (low/mid clock for the first ~3us of a busy
        # stretch) on junk matmuls over memset data, starting ~0.5us in --
        # no DMA dependency, so the first projections run at full clock
        jsrc = wpool.tile([128, 128], F16, tag="jsrc")
        nc.gpsimd.memset(jsrc[:], 0.0)
        junk = ps_pr.tile([128, SQW], F32, tag="pr", name="junk")
        for _ in range(30):
            nc.tensor.matmul(junk[:, 0:128], jsrc[:], jsrc[:],
                             start=True, stop=True)

        def _g3_chain(g3, m):
            t = g3[m]["t"]
            tr = ps_pr.tile([128, SQW], F16, tag="pr", name=f"tr_{t}")
            nc.tensor.transpose(tr[:, 0:128], g3[m]["hsq"][:], ident_sb[:])
            hdst = headst_sb[:, t * 128:(t + 1) * 128]
            nc.scalar.copy(hdst, tr[:, 0:128])
            ot = outp.tile([128, D_M], F16, tag="ot", name=f"ot_{t}")
            for dmc in range(D_M // SQW):
                op = ps_pr.tile([128, SQW], F32, tag="pr", name=f"op_{t}_{dmc}")
                nc.tensor.matmul(
                    op[:],
                    headst_sb[:, t * 128:(t + 1) * 128],
                    wot_sb[:, dmc * SQW:(dmc + 1) * SQW],
                    start=True,
                    stop=True,
                )
                dst = ot[:, dmc * SQW:(dmc + 1) * SQW]
                if dmc % 2:
                    nc.scalar.copy(dst, op[:])
                else:
                    nc.vector.tensor_copy(dst, op[:])
            nc.sync.dma_start(out[t * 128:(t + 1) * 128, :], ot[:])

        # ---- compute stream ----
        # c=0: only K0 x Q0 feasible (4 ET tiles)
        project(tq[0], wq_sb, qpt_sb, 0, "pq0")
        project(k0a, wk_sb, kpt_sb, 0, "pk0a", lo=0, w=256)
        z_exp(0, 0)
        project(k0b, wk_sb, kpt_sb, 0, "pk0b", lo=256, w=256)
        z_exp(0, 1)
        # c=1  (z tiles needing only K1 go before pq1, which waits on Q1)
        project(tk[1], wk_sb, kpt_sb, 1, "pk1")
        z_exp(0, 2)
        z_exp(0, 3)
        project(tq[1], wq_sb, qpt_sb, 1, "pq1")
        for jj in range(4):
            z_exp(1, jj)
        project_v(tv[0], 0)
        # c=2
        project(tk[2], wk_sb, kpt_sb, 2, "pk2")
        z_exp(0, 4)
        z_exp(0, 5)
        z_exp(1, 4)
        z_exp(1, 5)
        project(tq[2], wq_sb, qpt_sb, 2, "pq2")
        for jj in range(6):
            z_exp(2, jj)
        project_v(tv[1], 1)
        # c=3
        project(tk[3], wk_sb, kpt_sb, 3, "pk3")
        z_exp(0, 6)
        z_exp(0, 7)
        z_exp(1, 6)
        z_exp(1, 7)
        project_v(tv[2], 2)
        z_exp(2, 6)
        z_exp(2, 7)
        project(tq[3], wq_sb, qpt_sb, 3, "pq3")
        project_v(tv[3], 3)
        # group 3's z/exp stream with groups 0-2's finish units interleaved
        # (the units' AV runs become runnable as those groups' last exps land)
        units = [(g, m) for g in (0, 1, 2) for m in range(4)]
        ui = 0
        for jj in range(NJJ):
            z_exp(3, jj)
            for _ in range(2 if 2 <= jj < 6 else 1):
                if ui < len(units):
                    unit(*units[ui])
                    ui += 1
        while ui < len(units):
            unit(*units[ui])
            ui += 1
        # group 3 runs stage-pipelined: by the time its last exp lands only
        # the final 2 AV matmuls of each run remain, so the per-unit work is
        # almost pure cross-engine chain latency -- overlap the stages of
        # consecutive units instead of running each unit end-to-end.
        g3 = {}
        for m in range(4):
            t = 12 + m
            g3[m] = dict(t=t)
            # PE: AV runs (slot for m waits muls of m-1, emitted below)
            g3[m]["accs"] = [av_run(3, h, m) for h in range(HPC)]
            # DVE: recips; ACT: muls into hsq
            hsq = hsqp.tile([128, HD], F16, tag="hsq", name=f"hsq_{t}")
            g3[m]["hsq"] = hsq
            for h in range(HPC):
                rec = smalls.tile([128, 1], F32, tag="rec", name=f"rec_{t}_{h}")
                nc.vector.reciprocal(rec[:], g3[m]["accs"][h][:, 64:65])
                nc.scalar.mul(hsq[:, h * 64:(h + 1) * 64],
                              g3[m]["accs"][h][:, 0:64], rec[:])
            # previous unit's transpose/outproj interleave here so its ACT/DVE
            # stages overlap this unit's AV+normalize
            if m > 0:
                _g3_chain(g3, m - 1)
        _g3_chain(g3, 3)
def _build_nc():
    nc = bacc.Bacc("TRN2", target_bir_lowering=False, debug=False,
                   num_devices=NCORES)
    aps = {
        "qt": nc.dram_tensor("qt", [D_X, S], F16, kind="ExternalInput").ap(),
        "kt": nc.dram_tensor("kt", [D_X, S], F16, kind="ExternalInput").ap(),
        "vt": nc.dram_tensor("vt", [D_X, S], F16, kind="ExternalInput").ap(),
        "wq": nc.dram_tensor("wq", [128, D_X], F16, kind="ExternalInput").ap(),
        "wk": nc.dram_tensor("wk", [128, D_X], F16, kind="ExternalInput").ap(),
        "wv": nc.dram_tensor("wv", [128, D_X], F16, kind="ExternalInput").ap(),
        "wot": nc.dram_tensor("wot", [HD, D_M], F16, kind="ExternalInput").ap(),
        "ident": nc.dram_tensor("ident", [128, 128], F16, kind="ExternalInput").ap(),
        "out": nc.dram_tensor("out", [S, D_M], F16, kind="ExternalOutput").ap(),
    }
    with tile.TileContext(nc) as tc:
        with nc.allow_low_precision(reason="fp16 matmul/softmax pipeline"):
            _emit(tc, nc, aps)
    nc.compile()
    return nc


def kernel(**inputs):
    global LAST_EXEC_NS, _NC_CACHE
    Q = np.asarray(inputs["Q"], dtype=np.float32)
    K = np.asarray(inputs["K"], dtype=np.float32)
    V = np.asarray(inputs["V"], dtype=np.float32)
    W_q = np.asarray(inputs["W_q"], dtype=np.float32)
    W_k = np.asarray(inputs["W_k"], dtype=np.float32)
    W_v = np.asarray(inputs["W_v"], dtype=np.float32)
    W_o = np.asarray(inputs["W_o"], dtype=np.float32)

    def _pack_w(W, h0):
        # device SBUF layout [p, c*128+f] = W[c*128+p, f]; full-rate DMA rows
        w = np.concatenate([W[h0 + i] for i in range(HPC)], axis=1)  # (D_X, HD)
        return np.ascontiguousarray(
            w.reshape(NXC, 128, HD).transpose(1, 0, 2).reshape(128, NXC * HD)
            .astype(np.float16))

    QT = np.ascontiguousarray(Q.T.astype(np.float16))
    KT = np.ascontiguousarray(K.T.astype(np.float16))
    VT = np.ascontiguousarray(V.T.astype(np.float16))
    ident = np.eye(128, dtype=np.float16)
    in_maps = []
    for c in range(NCORES):
        h0 = HPC * c
        in_maps.append({
            "qt": QT, "kt": KT, "vt": VT,
            "wq": _pack_w(W_q, h0), "wk": _pack_w(W_k, h0),
            "wv": _pack_w(W_v, h0),
            "wot": np.ascontiguousarray(
                W_o[:, c * HD:(c + 1) * HD].T.astype(np.float16)),
            "ident": ident,
        })

    if _NC_CACHE is None:
        _NC_CACHE = _build_nc()
    nc = _NC_CACHE

    trace = bool(os.environ.get("MHA_TRACE"))
    res = None
    if trace:
        try:
            res = run_bass_kernel_spmd(nc, in_maps, list(range(NCORES)),
                                       trace=True)
        except Exception as e:  # profiling infra unavailable -> run untraced
            print(f"[kernel] traced run failed ({e!r}); falling back")
            res = None
    if res is None:
        res = run_bass_kernel_spmd(nc, in_maps, list(range(NCORES)))

    LAST_EXEC_NS = getattr(res, "exec_time_ns", None)

    out = np.zeros((S, D_M), np.float32)
    for r in res.results:
        out += r["out"].astype(np.float32)
    return out


# revision 14
# speedup vs baseline: 1.0453x; 1.0324x over previous
"""Multi-head attention (16 heads, S=2048, d_model=1024, d_head=64) on 8 TRN2
NeuronCores, tensor-parallel over heads (2 heads per core).

Restructured from the 120us baseline around the TimelineSim cost model, where
matmul cost = output-free-size rows (K and M are free):

  * AV matmuls run transposed: out[sq=128, dv+1=65] with the exp tile as the
    stationary operand, 65 rows per accumulation step instead of 512 -> PE
    drops from 196k to 166k rows (~82us -> ~69us busy).
  * softmax normalize becomes a per-partition tensor_scalar (denominator is
    column 64 of the accumulator) -- no gpsimd broadcast.
  * z PSUM tiles are [128, 1024] (2 banks); exp runs 64x instead of 128x,
    halving the fixed per-instruction ACT overhead (~81us -> ~66us busy).
  * everything 2-byte: fp16 in/out of every matmul, exp computes
    exp(z/8 - 4) so the scores fit fp16 (max z/8 ~ 11.9), the bias cancels
    in the normalize. Output ships fp16 (half the out DMA), host sums in f32.
  * heads come out of AV as [sq, dv]; a 16x[128,128] PE transpose (+DVE copy)
    restores [hd, sq] for the output projection.

Schedule: inputs stream K0,Q0,K1,Q1,V0,K2,Q2,V1,K3,Q3,V2,V3 so the exp
stream (the ~66us ACT critical path, the pacer) starts by ~7us and never
starves. z/exp tiles are emitted greedily (group-ascending); group 0's AV
rides inline behind the V projections; groups 1-3 accumulate at the end of
the stream in group order, each followed by its normalize/transpose/outproj/
DMA chain so only group 3's chain is a tail. PSUM: 2x[128,1024] z +
2x[128,260] AV accumulators + 2x[128,512] proj/outproj = exactly 8 banks.
"""

import os

import numpy as np

import concourse.bass as bass
import concourse.tile as tile
from concourse import bacc, mybir
from concourse.bass_utils import run_bass_kernel_spmd

HEADS, D_K, D_V, D_X, D_M, S = 16, 64, 64, 1024, 1024, 2048
NCORES = 8
HPC = HEADS // NCORES          # heads per core
HD = HPC * D_K                 # 128: stacked head dim per core
SQW = 512                      # sq group width
NSQ = S // SQW                 # 4 groups
SKW = 128                      # sk chunk width (partition dim)
NSK = S // SKW                 # 16
NXC = D_X // 128               # 8 contraction chunks for projections
NJJ = NSK // 2                 # 8 skc-pairs (one [128,1024] z tile each)

F32 = mybir.dt.float32
F16 = mybir.dt.float16
EXP = mybir.ActivationFunctionType.Exp

LAST_EXEC_NS = None
_NC_CACHE = None


def _emit(tc, nc, aps):
    from contextlib import ExitStack

    qt, kt, vt, wq, wk, wv, wot, ident, out = (
        aps["qt"], aps["kt"], aps["vt"], aps["wq"], aps["wk"], aps["wv"],
        aps["wot"], aps["ident"], aps["out"],
    )

    with ExitStack() as ctx:
        wpool = ctx.enter_context(tc.tile_pool(name="weights", bufs=1))
        proj = ctx.enter_context(tc.tile_pool(name="proj", bufs=1))
        inp = ctx.enter_context(tc.tile_pool(name="inp", bufs=6))
        etp = ctx.enter_context(tc.tile_pool(name="et", bufs=52))
        hsqp = ctx.enter_context(tc.tile_pool(name="hsq", bufs=8))
        outp = ctx.enter_context(tc.tile_pool(name="outs", bufs=4))
        smalls = ctx.enter_context(tc.tile_pool(name="smalls", bufs=4))
        ps_z = ctx.enter_context(tc.tile_pool(name="ps_z", bufs=2, space="PSUM"))
        ps_av = ctx.enter_context(tc.tile_pool(name="ps_av", bufs=2, space="PSUM"))
        ps_pr = ctx.enter_context(tc.tile_pool(name="ps_pr", bufs=2, space="PSUM"))

        # ---- persistent SBUF tensors ----
        wq_sb = wpool.tile([128, D_X], F16, tag="wq")     # (xc p) stacked chunks
        wk_sb = wpool.tile([128, D_X], F16, tag="wk")
        wv_sb = wpool.tile([128, D_X], F16, tag="wv")
        wot_sb = wpool.tile([HD, D_M], F16, tag="wot")
        ident_sb = wpool.tile([128, 128], F16, tag="ident")
        qpt_sb = proj.tile([HD, S], F16, tag="qpt")
        kpt_sb = proj.tile([HD, S], F16, tag="kpt")
        # VpAug: per (h, skc) a (128 sk, 65) block: cols 0-63 = Vp, col 64 = 1
        vpa_sb = proj.tile([128, HPC * NSK * 65], F16, tag="vpa")
        headst_sb = proj.tile([HD, S], F16, tag="headst")

        def load_w(w_dram, w_sb):
            nc.sync.dma_start(w_sb[:], w_dram)

        def load_chunk(tt_dram, c, name, lo=0, w=SQW, tag="inp"):
            """One DMA: all 8 xc strips of cols [c*512+lo, +w) -> (128, 8, w)."""
            t = inp.tile([128, NXC, w], F16, tag=tag, name=name,
                         bufs=2 if tag == "inp0" else None)
            nc.sync.dma_start(
                t[:],
                tt_dram.rearrange("(xc p) s -> p xc s", p=128)[
                    :, :, c * SQW + lo:c * SQW + lo + w
                ],
            )
            return t

        def project(t, w_sb, dst_sb, c, name, lo=0, w=SQW):
            """dst_sb[:, c*512+lo : +w] = W.T @ X.T chunk cols (fp16)."""
            ps = ps_pr.tile([128, w], F32, tag="pr", name=name,
                            padded_shape=[128, SQW])
            for xc in range(NXC):
                nc.tensor.matmul(
                    ps[:],
                    w_sb[:, xc * 128:(xc + 1) * 128],
                    t[:, xc, :],
                    start=(xc == 0),
                    stop=(xc == NXC - 1),
                )
            nc.vector.tensor_copy(
                dst_sb[:, c * SQW + lo:c * SQW + lo + w], ps[:])

        def project_v(t, c):
            """VpAug sk-chunks for 512-chunk c: Vp = VT_chunk.T @ Wv directly
            in (sk, hd) layout."""
            for j in range(SQW // SKW):
                skc = c * (SQW // SKW) + j
                ps = ps_pr.tile([128, HD], F32, tag="pr", name=f"vp_{skc}",
                                padded_shape=[128, SQW])
                for xc in range(NXC):
                    nc.tensor.matmul(
                        ps[:],
                        t[:, xc, j * SKW:(j + 1) * SKW],
                        wv_sb[:, xc * 128:(xc + 1) * 128],
                        start=(xc == 0),
                        stop=(xc == NXC - 1),
                    )
                for h in range(HPC):
                    base = (h * NSK + skc) * 65
                    nc.vector.tensor_copy(
                        vpa_sb[:, base:base + 64],
                        ps[:, h * 64:(h + 1) * 64],
                    )

        ets = {}  # (h, g, jj) -> ET tile awaiting its AV matmuls

        def z_exp(g, jj):
            """Per head: one [128,1024] z tile (skc pair 2jj,2jj+1) + exp.

            exp(z/8 - 4): the -4 bias keeps the scores in fp16 range
            (max z/8 ~ 11.9 -> e^7.9 = 2.7e3) and cancels in the normalize.
            """
            for h in range(HPC):
                z_ps = ps_z.tile([128, 2 * SQW], F32, tag="z",
                                 name=f"z_{h}_{g}_{jj}")
                for half in range(2):
                    skc = 2 * jj + half
                    nc.tensor.matmul(
                        z_ps[:, half * SQW:(half + 1) * SQW],
                        kpt_sb[h * 64:(h + 1) * 64, skc * SKW:(skc + 1) * SKW],
                        qpt_sb[h * 64:(h + 1) * 64, g * SQW:(g + 1) * SQW],
                        start=True,
                        stop=True,
                    )
                et = etp.tile([128, 2 * SQW], F16, tag="et",
                              name=f"et_{h}_{g}_{jj}")
                nc.scalar.activation(et[:], z_ps[:], EXP,
                                     scale=1.0 / 8.0, bias=bias_sb[:])
                ets[(h, g, jj)] = et

        # PSUM accumulation groups must be contiguous per bank (interleaved
        # start/stop groups at different offsets in one bank corrupt the
        # result), so AV runs as per-(h, sq-128-subtile) bursts of 16
        # back-to-back matmuls, one bank each, after the group's exps.
        def av_run(g, h, m):
            acc = ps_av.tile([128, 65], F32, tag="av", name=f"av_{g}_{h}_{m}")
            for jj in range(NJJ):
                et = ets[(h, g, jj)]
                for half in range(2):
                    skc = 2 * jj + half
                    vb = (h * NSK + skc) * 65
                    nc.tensor.matmul(
                        acc[:],
                        et[:, half * SQW + m * 128:half * SQW + (m + 1) * 128],
                        vpa_sb[:, vb:vb + 65],
                        start=(skc == 0),
                        stop=(skc == NSK - 1),
                    )
            return acc

        def unit(g, m, tail=False):
            """One sq-128 tile end to end: 2 AV runs -> normalize ->
            transpose -> output projection -> DMA.

            Mid-stream (ACT busy with exps) everything non-PE runs on DVE;
            in the tail (ACT idle) the muls/copies shift to ACT so the
            serial PE<->DVE chain shortens.
            """
            t = g * 4 + m
            accs = [av_run(g, h, m) for h in range(HPC)]
            hsq = hsqp.tile([128, HD], F16, tag="hsq", name=f"hsq_{t}")
            for h in range(HPC):
                rec = smalls.tile([128, 1], F32, tag="rec", name=f"rec_{t}_{h}")
                nc.vector.reciprocal(rec[:], accs[h][:, 64:65])
                dsth = hsq[:, h * 64:(h + 1) * 64]
                if tail:
                    nc.scalar.mul(dsth, accs[h][:, 0:64], rec[:])
                else:
                    nc.vector.tensor_scalar_mul(dsth, accs[h][:, 0:64], rec[:])
            tr = ps_pr.tile([128, SQW], F16, tag="pr", name=f"tr_{t}")
            nc.tensor.transpose(tr[:, 0:128], hsq[:], ident_sb[:])
            hdst = headst_sb[:, t * 128:(t + 1) * 128]
            if tail:
                nc.scalar.copy(hdst, tr[:, 0:128])
            else:
                nc.vector.tensor_copy(hdst, tr[:, 0:128])
            ot = outp.tile([128, D_M], F16, tag="ot", name=f"ot_{t}")
            for dmc in range(D_M // SQW):
                op = ps_pr.tile([128, SQW], F32, tag="pr", name=f"op_{t}_{dmc}")
                nc.tensor.matmul(
                    op[:],
                    headst_sb[:, t * 128:(t + 1) * 128],
                    wot_sb[:, dmc * SQW:(dmc + 1) * SQW],
                    start=True,
                    stop=True,
                )
                dst = ot[:, dmc * SQW:(dmc + 1) * SQW]
                if tail and dmc % 2:
                    nc.scalar.copy(dst, op[:])
                else:
                    nc.vector.tensor_copy(dst, op[:])
            nc.sync.dma_start(out[t * 128:(t + 1) * 128, :], ot[:])

        # ---- DMA stream (SP queue, in order) ----
        # wk/wq first (small, full-rate), then Q0 and K0 in 256-halves: the
        # first z tile only needs kpt cols 0:256, so the exp stream starts
        # ~3.5us earlier than with a monolithic K0
        tk, tq, tv = {}, {}, {}
        load_w(wk, wk_sb)
        load_w(wq, wq_sb)
        tq[0] = load_chunk(qt, 0, "qc_0")
        k0a = load_chunk(kt, 0, "kc_0a", lo=0, w=256, tag="inp0")
        k0b = load_chunk(kt, 0, "kc_0b", lo=256, w=256, tag="inp0")
        nc.sync.dma_start(ident_sb[:], ident)
        tk[1] = load_chunk(kt, 1, "kc_1")
        tq[1] = load_chunk(qt, 1, "qc_1")
        load_w(wv, wv_sb)
        tv[0] = load_chunk(vt, 0, "vc_0")
        nc.sync.dma_start(wot_sb[:], wot)
        tk[2] = load_chunk(kt, 2, "kc_2")
        tq[2] = load_chunk(qt, 2, "qc_2")
        tv[1] = load_chunk(vt, 1, "vc_1")
        tk[3] = load_chunk(kt, 3, "kc_3")
        tq[3] = load_chunk(qt, 3, "qc_3")
        tv[2] = load_chunk(vt, 2, "vc_2")
        tv[3] = load_chunk(vt, 3, "vc_3")

        # ones column of VpAug via gpsimd memset (no DMA needed)
        nc.gpsimd.memset(
            vpa_sb[:].rearrange("p (c f) -> p c f", f=65)[:, :, 64:65], 1.0)

        # exp bias constant (-4) as a per-partition scalar AP
        bias_sb = wpool.tile([128, 1], F32, tag="bias")
        nc.gpsimd.memset(bias_sb[:], -4.0)

        # absorb the 1.3us exp table load inside the initial DMA window
        warm = smalls.tile([128, 1], F32, tag="warm")
        nc.scalar.activation(warm[:], bias_sb[:], EXP, scale=1.0)

        # burn the PE pstate ramp (low/mid clock for the first ~3us of a busy
        # stretch) on junk matmuls over memset data, starting ~0.5us in --
        # no DMA dependency, so the first projections run at full clock
        jsrc = wpool.tile([128, 128], F16, tag="jsrc")
        nc.gpsimd.memset(jsrc[:], 0.0)
        junk = ps_pr.tile([128, SQW], F32, tag="pr", name="junk")
        for _ in range(30):
            nc.tensor.matmul(junk[:, 0:128], jsrc[:], jsrc[:],
                             start=True, stop=True)

        def _g3_chain(g3, m):
            t = g3[m]["t"]
            tr = ps_pr.tile([128, SQW], F16, tag="pr", name=f"tr_{t}")
            nc.tensor.transpose(tr[:, 0:128], g3[m]["hsq"][:], ident_sb[:])
            hdst = headst_sb[:, t * 128:(t + 1) * 128]
            nc.scalar.copy(hdst, tr[:, 0:128])
            ot = outp.tile([128, D_M], F16, tag="ot", name=f"ot_{t}")
            for dmc in range(D_M // SQW):
                op = ps_pr.tile([128, SQW], F32, tag="pr", name=f"op_{t}_{dmc}")
                nc.tensor.matmul(
                    op[:],
                    headst_sb[:, t * 128:(t + 1) * 128],
                    wot_sb[:, dmc * SQW:(dmc + 1) * SQW],
                    start=True,
                    stop=True,
                )
                dst = ot[:, dmc * SQW:(dmc + 1) * SQW]
                if dmc % 2:
                    nc.scalar.copy(dst, op[:])
                else:
                    nc.vector.tensor_copy(dst, op[:])
            nc.sync.dma_start(out[t * 128:(t + 1) * 128, :], ot[:])

        # ---- compute stream ----
        # c=0: only K0 x Q0 feasible (4 ET tiles)
        project(tq[0], wq_sb, qpt_sb, 0, "pq0")
        project(k0a, wk_sb, kpt_sb, 0, "pk0a", lo=0, w=256)
        z_exp(0, 0)
        project(k0b, wk_sb, kpt_sb, 0, "pk0b", lo=256, w=256)
        z_exp(0, 1)
        # c=1  (z tiles needing only K1 go before pq1, which waits on Q1)
        project(tk[1], wk_sb, kpt_sb, 1, "pk1")
        z_exp(0, 2)
        z_exp(0, 3)
        project(tq[1], wq_sb, qpt_sb, 1, "pq1")
        for jj in range(4):
            z_exp(1, jj)
        project_v(tv[0], 0)
        # c=2
        project(tk[2], wk_sb, kpt_sb, 2, "pk2")
        z_exp(0, 4)
        z_exp(0, 5)
        z_exp(1, 4)
        z_exp(1, 5)
        project(tq[2], wq_sb, qpt_sb, 2, "pq2")
        for jj in range(6):
            z_exp(2, jj)
        project_v(tv[1], 1)
        # c=3
        project(tk[3], wk_sb, kpt_sb, 3, "pk3")
        z_exp(0, 6)
        z_exp(0, 7)
        z_exp(1, 6)
        z_exp(1, 7)
        project_v(tv[2], 2)
        z_exp(2, 6)
        z_exp(2, 7)
        project(tq[3], wq_sb, qpt_sb, 3, "pq3")
        project_v(tv[3], 3)
        # group 3's z/exp stream with groups 0-2's finish units interleaved.
        # group 0's exps all land before the first z(3) can even claim a PSUM
        # slot, so its 4 units go first -- anything behind z(3,0) in the PE
        # FIFO would stall on the slot wait.
        for m in range(4):
            unit(0, m)
        units = [(g, m) for g in (1, 2) for m in range(4)]
        ui = 0
        for jj in range(NJJ):
            z_exp(3, jj)
            if ui < len(units):
                unit(*units[ui])
                ui += 1
        # group 3 runs stage-pipelined: by the time its last exp lands only
        # the final 2 AV matmuls of each run remain, so the per-unit work is
        # almost pure cross-engine chain latency -- overlap the stages of
        # consecutive units instead of running each unit end-to-end.
        g3 = {}
        for m in range(4):
            t = 12 + m
            g3[m] = dict(t=t)
            # PE: AV runs (slot for m waits muls of m-1, emitted below)
            g3[m]["accs"] = [av_run(3, h, m) for h in range(HPC)]
            # DVE: recips; ACT: muls into hsq
            hsq = hsqp.tile([128, HD], F16, tag="hsq", name=f"hsq_{t}")
            g3[m]["hsq"] = hsq
            for h in range(HPC):
                rec = smalls.tile([128, 1], F32, tag="rec", name=f"rec_{t}_{h}")
                nc.vector.reciprocal(rec[:], g3[m]["accs"][h][:, 64:65])
                nc.scalar.mul(hsq[:, h * 64:(h + 1) * 64],
                              g3[m]["accs"][h][:, 0:64], rec[:])
            # previous unit's transpose/outproj interleave here so its ACT/DVE
            # stages overlap this unit's AV+normalize
            if m > 0:
                _g3_chain(g3, m - 1)
        _g3_chain(g3, 3)
def _build_nc():
    nc = bacc.Bacc("TRN2", target_bir_lowering=False, debug=False,
                   num_devices=NCORES)
    aps = {
        "qt": nc.dram_tensor("qt", [D_X, S], F16, kind="ExternalInput").ap(),
        "kt": nc.dram_tensor("kt", [D_X, S], F16, kind="ExternalInput").ap(),
        "vt": nc.dram_tensor("vt", [D_X, S], F16, kind="ExternalInput").ap(),
        "wq": nc.dram_tensor("wq", [128, D_X], F16, kind="ExternalInput").ap(),
        "wk": nc.dram_tensor("wk", [128, D_X], F16, kind="ExternalInput").ap(),
        "wv": nc.dram_tensor("wv", [128, D_X], F16, kind="ExternalInput").ap(),
        "wot": nc.dram_tensor("wot", [HD, D_M], F16, kind="ExternalInput").ap(),
        "ident": nc.dram_tensor("ident", [128, 128], F16, kind="ExternalInput").ap(),
        "out": nc.dram_tensor("out", [S, D_M], F16, kind="ExternalOutput").ap(),
    }
    with tile.TileContext(nc) as tc:
        with nc.allow_low_precision(reason="fp16 matmul/softmax pipeline"):
            _emit(tc, nc, aps)
    nc.compile()
    return nc


def kernel(**inputs):
    global LAST_EXEC_NS, _NC_CACHE
    Q = np.asarray(inputs["Q"], dtype=np.float32)
    K = np.asarray(inputs["K"], dtype=np.float32)
    V = np.asarray(inputs["V"], dtype=np.float32)
    W_q = np.asarray(inputs["W_q"], dtype=np.float32)
    W_k = np.asarray(inputs["W_k"], dtype=np.float32)
    W_v = np.asarray(inputs["W_v"], dtype=np.float32)
    W_o = np.asarray(inputs["W_o"], dtype=np.float32)

    def _pack_w(W, h0):
        # device SBUF layout [p, c*128+f] = W[c*128+p, f]; full-rate DMA rows
        w = np.concatenate([W[h0 + i] for i in range(HPC)], axis=1)  # (D_X, HD)
        return np.ascontiguousarray(
            w.reshape(NXC, 128, HD).transpose(1, 0, 2).reshape(128, NXC * HD)
            .astype(np.float16))

    QT = np.ascontiguousarray(Q.T.astype(np.float16))
    KT = np.ascontiguousarray(K.T.astype(np.float16))
    VT = np.ascontiguousarray(V.T.astype(np.float16))
    ident = np.eye(128, dtype=np.float16)
    in_maps = []
    for c in range(NCORES):
        h0 = HPC * c
        in_maps.append({
            "qt": QT, "kt": KT, "vt": VT,
            "wq": _pack_w(W_q, h0), "wk": _pack_w(W_k, h0),
            "wv": _pack_w(W_v, h0),
            "wot": np.ascontiguousarray(
                W_o[:, c * HD:(c + 1) * HD].T.astype(np.float16)),
            "ident": ident,
        })

    if _NC_CACHE is None:
        _NC_CACHE = _build_nc()
    nc = _NC_CACHE

    trace = bool(os.environ.get("MHA_TRACE"))
    res = None
    if trace:
        try:
            res = run_bass_kernel_spmd(nc, in_maps, list(range(NCORES)),
                                       trace=True)
        except Exception as e:  # profiling infra unavailable -> run untraced
            print(f"[kernel] traced run failed ({e!r}); falling back")
            res = None
    if res is None:
        res = run_bass_kernel_spmd(nc, in_maps, list(range(NCORES)))

    LAST_EXEC_NS = getattr(res, "exec_time_ns", None)

    out = np.zeros((S, D_M), np.float32)
    for r in res.results:
        out += r["out"].astype(np.float32)
    return out
